# revision 1
# baseline (speedup 1.0000x reference)
"""DTW frames layer on 8 Trainium2 NeuronCores.

Reference computation (per (n, k) problem):
    cost[p, w] = max(0, ||x[n, :, w] - patts[k, :, p]||^2)          (P=32, W=128)
    dtw[0, w]  = cumsum_w cost[0, w]
    dtw[p, 0]  = cumsum_p cost[p, 0]
    dtw[p, w]  = cost[p, w] + min(dtw[p, w-1], dtw[p-1, w-1], dtw[p-1, w])
    out        = sqrt(dtw[:, -32:]) / 32

Strategy:
  - Data-parallel over batch n: each of the 8 cores owns n_loc = 8 rows of x,
    patterns replicated. Per core, two problem tiles of 128 partitions each
    (4 n x 32 k).
  - Cost matrix via one augmented K=10 fp32 matmul per (row-quad m, n-chunk):
    lhsT columns ordered p-major so PSUM partitions are (p%4, k); lhsT rows =
    [-2*patts[d], ||patt||^2, 1], rhs rows = [x[d], 1, ||x||^2], so PSUM is
    the cost before clamping; ReLU on eviction applies max(0, .) and packs
    row-quads into mm_big[128, (nn4 m8 w128)].
  - The (p%4, k) -> (nn, k) partition regroup into scan layout C2 is pure
    data movement with 2 KB contiguous runs on both sides; it runs as 64
    ACT-issued SBUF->SBUF DMAs of [32 partitions x 2 KB] (the naive permute
    had 512 B runs and was descriptor-rate bound at ~90 us; GPSIMD copies
    measured ~2 us each).
  - DTW row recurrence on the DVE tensor_tensor_scan instruction:
    state = (m[t] min state) add c[t], one instruction per table row, where
    m[t] = min(dtw[p-1, t-1], dtw[p-1, t]) is one shifted tensor_tensor min.
    Column 0 of the m operand stays at +BIG so element 0 of each scan is
    initial + cost (the first-column cumsum), initial = prev row's column 0.
    All operands are dense 2D slices - multi-dim APs measurably slow DVE.
  - Rows land in a persistent D[128, P*W] buffer; one batched Sqrt
    activation per tile computes sqrt(dtw/1024) = sqrt(dtw)/32 on the last
    32 columns of every row.
  - Every ISA instruction has ONE sync-wait slot, so the build keeps each
    instruction's emitted waits to a single semaphore: tiny same-engine
    "wait absorber" ops soak up producer waits ahead of DMA triggers and
    cross-engine consumers, and a tail nop-chain feeds every proc's final
    tick into the sync sequencer so the kernel-tail drain elides its
    (single-slot) wait list.
"""

import numpy as np

import concourse.bass as bass
import concourse.mybir as mybir
import concourse.tile as tile
from concourse.bass_utils import run_bass_kernel_spmd

N, D, W = 64, 8, 128      # x: (N, D, W)
K, P = 32, 32             # patts: (K, D, P)
WO = 32                   # output keeps last WO columns of the DTW table
NCORES = 8
NLOC = N // NCORES        # 8 batch rows per core
NT = 2                    # problem tiles per core: (4 n x 32 k) = 128 partitions
KAUG = D + 2              # augmented contraction dim
BIG = 1e30

f32 = mybir.dt.float32


def _rowmap(p: int) -> int:
    """C2 stores row p at index (p%4)*8 + p//4 (copy-contiguity order)."""
    return (p % 4) * 8 + p // 4


def build_program() -> bass.Bass:
    from concourse.tile import add_dep_helper

    nc = bass.Bass()
    inp_d = nc.dram_tensor("inp", (KAUG, K * P + NLOC * W), f32, kind="ExternalInput")
    out_d = nc.dram_tensor("out", (NLOC, K, P, WO), f32, kind="ExternalOutput")

    with tile.TileContext(nc) as tc:
        with (
            tc.tile_pool(name="consts", bufs=1) as consts,
            tc.tile_pool(name="psum", bufs=8, space="PSUM") as psum_pool,
            tc.tile_pool(name="mmb", bufs=1) as mmb_pool,
            tc.tile_pool(name="cbuf", bufs=1) as c_pool,
            tc.tile_pool(name="dbuf", bufs=1) as d_pool,
            tc.tile_pool(name="mbuf", bufs=2) as m_pool,
            tc.tile_pool(name="obuf", bufs=2) as o_pool,
        ):
            inp_s = consts.tile([KAUG, K * P + NLOC * W], f32)
            nc.sync.dma_start(out=inp_s, in_=inp_d[:, :])
            lhs_s = inp_s[:, 0:K * P]
            rhs_s = inp_s[:, K * P:K * P + NLOC * W]
            facta = consts.tile([1, 1], f32)
            factd = [
                consts.tile([1, 1], f32, name=f"factd{i}", tag=f"factd{i}")
                for i in range(8 * 2 * NT)
            ]

            # mm_big[t]: cost rows in matmul partition order (p%4, k), free
            # flat (nn, m, w). C2[t]: scan layout, partitions (nn, k), rows
            # in _rowmap order. D[t]: the DTW table, rows in true order.
            mm_big = [
                mmb_pool.tile([128, 4 * 8 * W], f32, tag=f"mmb{t}", name=f"mmb{t}")
                for t in range(NT)
            ]
            C2 = [
                c_pool.tile([128, P * W], f32, tag=f"C{t}", name=f"C{t}")
                for t in range(NT)
            ]

            last_mm = None
            relu = {}
            for m in range(8):           # row-quad: p in {4m .. 4m+3}
                for t in range(NT):      # n-chunk: n in {4t .. 4t+3}
                    ps = psum_pool.tile([128, 512], f32)
                    last_mm = nc.tensor.matmul(
                        ps,
                        lhs_s[:, m * 128:(m + 1) * 128],
                        rhs_s[:, t * 512:(t + 1) * 512],
                        start=True,
                        stop=True,
                    )
                    # strided out: per nn a 128-elem w-run at stride 1024
                    mmv = mm_big[t].rearrange("q (nn m w) -> q nn m w", nn=4, m=8)
                    relu[(m, t)] = nc.scalar.activation(
                        mmv[:, :, m, :], ps,
                        mybir.ActivationFunctionType.Relu,
                    )

            # Partition-block regroup (pp, k) -> (nn, k): 64 sync-issued DMAs
            # of [32 partitions x 2 KB contiguous] each. A sync-sequencer nop
            # per (mq, t) soaks the relu wait so every DMA trigger carries
            # only its queue-predecessor wait (one ISA slot); keeping the
            # triggers off the ACT sequencer leaves it free for relus/sqrt.
            dmas = []                    # in issue order, for queue tracking
            copies = {}
            for mq in range(2):
                for t in range(NT):
                    fence = nc.scalar.activation(
                        facta, mm_big[t][0:1, 0:1],
                        mybir.ActivationFunctionType.Copy,
                    )
                    for mr in range(4):
                        add_dep_helper(
                            fence.ins, relu[(mq * 4 + mr, t)].ins, sync=True,
                            reason="ACT absorbs relu batch",
                        )
                    for nn in range(4):
                        for pp in range(4):
                            dma = nc.scalar.dma_start(
                                out=C2[t][nn * 32:(nn + 1) * 32,
                                          (pp * 8 + mq * 4) * W:
                                          (pp * 8 + mq * 4 + 4) * W],
                                in_=mm_big[t][pp * 32:(pp + 1) * 32,
                                              nn * 1024 + mq * 512:
                                              nn * 1024 + (mq + 1) * 512],
                            )
                            add_dep_helper(
                                dma.ins, fence.ins, sync=False,
                                reason="regroup DMA after sync absorber",
                            )
                            copies[(t, mq, nn, pp)] = dma
                            dmas.append(dma)

            # DTW scans: the two tiles' chains are emitted row-interleaved so
            # the in-order DVE fills one chain's dependency-turnaround gap
            # with the other chain's op.
            mts, Dts, last_scan = [], [], [None] * NT
            for t in range(NT):
                mt = m_pool.tile([128, W], f32, tag=f"mt{t}", name=f"mt{t}")
                nc.vector.memset(mt, BIG)
                mts.append(mt)
                Dts.append(
                    d_pool.tile([128, P * W], f32, tag=f"D{t}", name=f"D{t}")
                )
            first_abs = {}
            for p in range(P):
                for t in range(NT):
                    mt, Dt = mts[t], Dts[t]
                    if p % 16 == 0:
                        # DVE wait absorbers: the 16 regroup DMAs feeding
                        # rows [p, p+16) span all 8 HWDGE queues; one
                        # single-wait fence per queue-max DMA (the last 8
                        # of the batch) covers them all.
                        mq = p // 16
                        batch = [copies[(t, mq, nn, pp)]
                                 for nn in range(4) for pp in range(4)]
                        for j, bd in enumerate(batch[8:]):
                            df = nc.vector.tensor_copy(
                                factd[(t * 2 + mq) * 8 + j],
                                C2[t][0:1, mq * 4 * W:mq * 4 * W + 1],
                            )
                            add_dep_helper(
                                df.ins, bd.ins, sync=True,
                                reason="DVE absorbs regroup queue",
                            )
                            first_abs[t] = df
                    r = _rowmap(p)
                    cr = C2[t][:, r * W:(r + 1) * W]
                    if p == 0:
                        scan = nc.vector.tensor_tensor_scan(
                            Dt[:, 0:W], mt, cr, 0.0,
                            mybir.AluOpType.min, mybir.AluOpType.add,
                        )
                    else:
                        o = (p - 1) * W
                        nc.vector.tensor_tensor(
                            mt[:, 1:W], Dt[:, o:o + W - 1], Dt[:, o + 1:o + W],
                            mybir.AluOpType.min,
                        )
                        scan = nc.vector.tensor_tensor_scan(
                            Dt[:, p * W:(p + 1) * W], mt, cr, Dt[:, o:o + 1],
                            mybir.AluOpType.min, mybir.AluOpType.add,
                        )
                    if p % 16 == 0:
                        add_dep_helper(
                            scan.ins, first_abs[t].ins, sync=False,
                            reason="scan after DVE absorbers",
                        )
                    last_scan[t] = scan

            last_ofence = None
            odmas = []
            for t in range(NT):
                Dt = Dts[t]
                ot = o_pool.tile([128, P, WO], f32)
                dv = Dt.rearrange("q (p w) -> q p w", p=P)
                nc.scalar.activation(
                    ot[:, :, :], dv[:, :, W - WO:W],
                    mybir.ActivationFunctionType.Sqrt,
                    scale=1.0 / (P * P),
                )
                ofence = nc.scalar.activation(
                    facta, ot[0:1, P - 1, 0:1], mybir.ActivationFunctionType.Copy
                )
                last_ofence = ofence
                odma = nc.scalar.dma_start(
                    out=out_d[t * 4:(t + 1) * 4, :, :, :], in_=ot
                )
                add_dep_helper(
                    odma.ins, ofence.ins, sync=False,
                    reason="out DMA after ACT wait-absorber",
                )
                odmas.append(odma)

            # Tail: feed every proc's final tick into the sync sequencer so
            # the kernel-tail drain's single-slot wait list elides. The last
            # 8 regroup DMAs + the out DMAs cover every HWDGE queue's max.
            tail_deps = dmas[-8:] + odmas + [last_ofence, last_mm] + last_scan
            prev_nop = None
            for td in tail_deps:
                nop = nc.sync.nop()
                add_dep_helper(
                    nop.ins, td.ins, sync=True,
                    reason="drain pre-absorber: sync waits on proc tail",
                )
                if prev_nop is not None:
                    add_dep_helper(
                        nop.ins, prev_nop.ins, sync=False,
                        reason="keep nop chain ordered",
                    )
                prev_nop = nop
    return nc


def make_in_maps(x: np.ndarray, patts: np.ndarray) -> list[dict[str, np.ndarray]]:
    x = np.ascontiguousarray(x, dtype=np.float32)
    patts = np.ascontiguousarray(patts, dtype=np.float32)
    pf = patts.transpose(1, 2, 0).reshape(D, P * K)              # [d, (p k)]
    p2 = (patts * patts).sum(axis=1).T.reshape(1, P * K)         # [(p k)]
    ones_pk = np.ones((1, P * K), np.float32)
    lhs = np.concatenate([-2.0 * pf, p2, ones_pk], axis=0).astype(np.float32)

    in_maps = []
    for c in range(NCORES):
        xs = x[c * NLOC:(c + 1) * NLOC]                          # (8, 8, 128)
        xf = xs.transpose(1, 0, 2).reshape(D, NLOC * W)          # [d, (n w)]
        x2 = (xs * xs).sum(axis=1).reshape(1, NLOC * W)          # [(n w)]
        ones_nw = np.ones((1, NLOC * W), np.float32)
        rhs = np.concatenate([xf, ones_nw, x2], axis=0).astype(np.float32)
        in_maps.append({"inp": np.concatenate([lhs, rhs], axis=1)})
    return in_maps


_program_cache: bass.Bass | None = None


def kernel(x: np.ndarray, patts: np.ndarray) -> np.ndarray:
    global _program_cache
    if _program_cache is None:
        _program_cache = build_program()
    nc = _program_cache
    in_maps = make_in_maps(x, patts)
    res = run_bass_kernel_spmd(nc, in_maps, list(range(NCORES)))
    return np.concatenate([r["out"] for r in res.results], axis=0)


if __name__ == "__main__":
    rng = np.random.default_rng(0)
    x = rng.standard_normal((N, D, W), dtype=np.float32)
    patts = rng.standard_normal((K, D, P), dtype=np.float32)
    out = kernel(x, patts)
    print(out.shape, out.dtype)



# revision 18
# speedup vs baseline: 1.5137x; 1.5137x over previous
"""DTW frames layer on 8 Trainium2 NeuronCores.

Reference computation (per (n, k) problem):
    cost[p, w] = max(0, ||x[n, :, w] - patts[k, :, p]||^2)          (P=32, W=128)
    dtw[0, w]  = cumsum_w cost[0, w]
    dtw[p, 0]  = cumsum_p cost[p, 0]
    dtw[p, w]  = cost[p, w] + min(dtw[p, w-1], dtw[p-1, w-1], dtw[p-1, w])
    out        = sqrt(dtw[:, -32:]) / 32

Strategy (v2; baseline was 108.3us):
  - Data-parallel over batch n: each of 8 cores owns n_loc = 8 rows of x,
    patterns replicated. Per core, two problem tiles of 128 partitions
    (4 n x 32 k); tile t covers n = 4t..4t+3.
  - Cost via one augmented K=10 *fp16* matmul per (q, t): single HW pass
    (fp32 ran LOW/HIGH double-pumped at 2.1us/matmul; fp16 measures 427ns).
    lhs columns ordered so slice q holds patterns p == q (mod 8) with
    partition block b = p // 8. PSUM fp32 -> relu-evict into mm_big fp32
    with free layout (nn, q, w). Tile0's evicts split ACT/DVE to shorten
    the critical path to the first regroup DMA.
  - Regroup (b,k) -> (nn,k) partitions: 32 SBUF->SBUF DMAs (t, nn, b) of
    [32 parts x 4 KB] contiguous on both sides (the p = b*8+q column order
    makes each DMA cover 8 *consecutive* DTW rows, so DMA order matches
    scan consumption order). All triggers on the idle SP sequencer
    (~640ns each measured); the ACT-issued 64-trigger scheme of the
    baseline burned 38us of ACT sequencer time.
  - DTW rows in fp32 (fp16 does NOT speed tensor_tensor_scan - it is
    carry-bound at ~2 cycles/elem either way; measured 397ns vs 397ns).
    D rows are stored at stride W+1 with a BIG guard column before each
    row: the shifted-min tensor_tensor then produces m[0] = D[p-1,0]
    without a separate copy, and the scan's initial is the immediate BIG
    (elem0 = min(m[0], BIG) + c[0] = D[p-1,0] + c[0]), avoiding the
    per-scan initial-AP read (~85ns/scan).
  - Every ISA instruction has ONE sync-wait slot: regroup-DMA completion
    waits ride along on earlier DVE ops whose own deps are same-engine
    (manual add_dep_helper), with tiny tensor_copy absorbers only for the
    first group of each tile; SP nops absorb the relu-completion waits
    ahead of the DMA trigger batches; a tail nop-chain feeds every
    proc's final tick to the sync sequencer so the drain's wait elides.
"""

import numpy as np

import concourse.bass as bass
import concourse.mybir as mybir
import concourse.tile as tile
from concourse.bass_utils import run_bass_kernel_spmd

N, D, W = 64, 8, 128      # x: (N, D, W)
K, P = 32, 32             # patts: (K, D, P)
WO = 32                   # output keeps last WO columns of the DTW table
NCORES = 8
NLOC = N // NCORES        # 8 batch rows per core
NT = 2                    # problem tiles per core: (4 n x 32 k) = 128 partitions
KAUG = D + 2              # augmented contraction dim
BIG = 1e30
WG = W + 1                # row pitch in D (guard column + W data columns)

f32 = mybir.dt.float32
f16 = mybir.dt.float16


def build_program() -> bass.Bass:
    from concourse.tile import add_dep_helper

    nc = bass.Bass()
    inp_d = nc.dram_tensor("inp", (KAUG, K * P + NLOC * W), f16, kind="ExternalInput")
    out_d = nc.dram_tensor("out", (NLOC, K, P, WO), f32, kind="ExternalOutput")

    with tile.TileContext(nc) as tc:
        with (
            tc.tile_pool(name="consts", bufs=1) as consts,
            tc.tile_pool(name="psum", bufs=8, space="PSUM") as psum_pool,
            tc.tile_pool(name="work", bufs=1) as work,
        ):
            inp_s = consts.tile([KAUG, K * P + NLOC * W], f16)
            nc.sync.dma_start(out=inp_s, in_=inp_d[:, :])
            lhs_s = inp_s[:, 0:K * P]
            rhs_s = inp_s[:, K * P:K * P + NLOC * W]
            facta = consts.tile([1, 1], f32)
            factd = [
                consts.tile([1, 1], f32, name=f"factd{i}", tag=f"factd{i}")
                for i in range(4 * 4 * NT)
            ]

            mm_big = [
                work.tile([128, 4 * 8 * W], f32, tag=f"mmb{t}", name=f"mmb{t}")
                for t in range(NT)
            ]
            C2 = [
                work.tile([128, P * W], f32, tag=f"C{t}", name=f"C{t}")
                for t in range(NT)
            ]
            # D table: both tiles in ONE tile so future AP tricks can span
            # them; row p of tile t at cols t*P*WG + p*WG + 1 .. +W, guard
            # (BIG) at t*P*WG + p*WG.
            Dt = work.tile([128, NT * P * WG], f32, tag="D", name="D")
            mt_big = work.tile([128, W], f32, tag="mtb", name="mtb")
            mts = [
                work.tile([128, W], f32, tag=f"mt{t}", name=f"mt{t}")
                for t in range(NT)
            ]

            # Guard memsets on DVE so every TT's guard-read dep coalesces
            # with its same-engine row dep into one sem wait.
            nc.vector.memset(mt_big, BIG)
            for t in range(NT):
                gv = Dt[:, t * P * WG:(t + 1) * P * WG].rearrange(
                    "q (p w) -> q p w", p=P)
                nc.vector.memset(gv[:, :, 0:1], BIG)

            # --- matmuls + evicts: t0 q0..7 then t1 q0..7.  t0's evicts
            # split ACT (q0..3) / DVE (q4..7) to finish ~2.5us earlier.
            relu_acts = {t: [] for t in range(NT)}   # ACT evicts per tile
            relu_dves = {t: [] for t in range(NT)}
            last_mm = None
            act_fence = None

            def emit_tile_mms(t):
                nonlocal last_mm
                mmv = mm_big[t].rearrange("q (nn g w) -> q nn g w", nn=4, g=8)
                for q in range(8):
                    ps = psum_pool.tile([128, 512], f32)
                    last_mm = nc.tensor.matmul(
                        ps,
                        lhs_s[:, q * 128:(q + 1) * 128],
                        rhs_s[:, t * 512:(t + 1) * 512],
                        start=True,
                        stop=True,
                    )
                    if t == 0 and q >= 4:
                        ev = nc.vector.tensor_scalar_max(mmv[:, :, q, :], ps, 0.0)
                        relu_dves[t].append(ev)
                    else:
                        ev = nc.scalar.activation(
                            mmv[:, :, q, :], ps,
                            mybir.ActivationFunctionType.Relu,
                        )
                        relu_acts[t].append(ev)

            emit_tile_mms(0)
            emit_tile_mms(1)

            # --- regroup DMAs (t, nn, b): mm_big[t][b-block parts,
            # nn-block free] -> C2[t][nn-block parts, rows b*8..b*8+7].
            # Tile0's 16 trigger on the idle SP sequencer; the first one
            # sync-deps the ACT fence (drops its direct DVE-evict edges,
            # keeps one coalesced ACT wait), the second carries the single
            # coalesced DVE wait, later ones elide via SP's wait clock and
            # carry only HWDGE slot-reuse waits.  Tile1's 16 trigger on
            # gpsimd (SWDGE): a separate queue space, and the Pool engine
            # is otherwise idle; the first carries the coalesced ACT wait
            # for t1's evicts.
            def emit_group(t, b):
                eng = nc.sync if t == 0 else nc.gpsimd
                out = []
                for nn in range(4):
                    dma = eng.dma_start(
                        out=C2[t][nn * 32:(nn + 1) * 32,
                                  b * 8 * W:(b + 1) * 8 * W],
                        in_=mm_big[t][b * 32:(b + 1) * 32,
                                      nn * 8 * W:(nn + 1) * 8 * W],
                    )
                    out.append(dma)
                return out

            # SP pre-DMA reading a q7 (DVE-evicted) corner: carries the
            # single coalesced DVE wait and registers it in SP's wait
            # clock, so the real regroup DMAs' DVE-evict deps elide and
            # each carries at most the coalesced ACT wait / a slot wait.
            scrap = consts.tile([1, 64], f32, name="scrap", tag="scrap")
            nc.sync.dma_start(out=scrap, in_=mm_big[0][0:1, 7 * W:7 * W + 64])
            groups = {}
            for b in range(4):
                groups[(0, b)] = emit_group(0, b)
            for b in range(4):
                groups[(1, b)] = emit_group(1, b)
            all_dmas = [d for g in groups.values() for d in g]

            # --- DTW scans.  Schedule: t0 rows 0..7 solo, then lag-8
            # interleave, then t1 rows 24..31 solo.
            sched = [(0, p) for p in range(8)]
            for i in range(24):
                sched.append((1, i))
                sched.append((0, i + 8))
            sched += [(1, p) for p in range(24, 32)]

            nfact = 0
            last_scan = {}
            group_anchor = None

            for (t, p) in sched:
                base = t * P * WG + p * WG
                cr = C2[t][:, p * W:(p + 1) * W]
                if p % 8 == 0:
                    # 4 absorber copies, one per regroup DMA of this group
                    # (corner read -> auto-dep, one DMA-sem wait each);
                    # nosync edges chain them and anchor the first scan.
                    b = p // 8
                    prev_ab = None
                    for i in range(4):
                        ab = nc.vector.tensor_copy(
                            factd[nfact],
                            C2[t][i * 32:i * 32 + 1, b * 8 * W:b * 8 * W + 1],
                        )
                        nfact += 1
                        if prev_ab is not None:
                            add_dep_helper(ab.ins, prev_ab.ins, sync=False,
                                           reason="absorber chain order")
                        prev_ab = ab
                    group_anchor = prev_ab
                if p == 0:
                    scan = nc.vector.tensor_tensor_scan(
                        Dt[:, base + 1:base + 1 + W], mt_big, cr, 0.0,
                        mybir.AluOpType.min, mybir.AluOpType.add,
                    )
                else:
                    pbase = t * P * WG + (p - 1) * WG
                    mt = mts[t]
                    nc.vector.tensor_tensor(
                        mt, Dt[:, pbase:pbase + W],
                        Dt[:, pbase + 1:pbase + 1 + W],
                        mybir.AluOpType.min,
                    )
                    scan = nc.vector.tensor_tensor_scan(
                        Dt[:, base + 1:base + 1 + W], mt, cr, BIG,
                        mybir.AluOpType.min, mybir.AluOpType.add,
                    )
                if p % 8 == 0:
                    add_dep_helper(scan.ins, group_anchor.ins, sync=False,
                                   reason="scan after DVE absorbers")
                last_scan[t] = scan

            # --- sqrt + out DMA per tile (ACT), overlapped with the other
            # tile's remaining scans.
            odmas, ofences = [], []
            for t in range(NT):
                ot = work.tile([128, P, WO], f32, name=f"ot{t}", tag=f"ot{t}")
                dv = Dt[:, t * P * WG:(t + 1) * P * WG].rearrange(
                    "q (p w) -> q p w", p=P)
                nc.scalar.activation(
                    ot[:, :, :], dv[:, :, WG - WO:WG],
                    mybir.ActivationFunctionType.Sqrt,
                    scale=1.0 / (P * P),
                )
                ofence = nc.scalar.activation(
                    facta, ot[0:1, P - 1, 0:1],
                    mybir.ActivationFunctionType.Copy,
                )
                odma = nc.scalar.dma_start(
                    out=out_d[t * 4:(t + 1) * 4, :, :, :], in_=ot
                )
                add_dep_helper(odma.ins, ofence.ins, sync=False,
                               reason="out DMA after ACT wait-absorber")
                odmas.append(odma)
                ofences.append(ofence)

            # --- tail: feed every proc's final tick into the sync
            # sequencer so the kernel-tail drain's wait list elides.
            tail_deps = (all_dmas[8:16] + all_dmas[-8:] + odmas
                         + [ofences[-1], last_mm]
                         + [last_scan[t] for t in range(NT)])
            prev_nop = None
            for td in tail_deps:
                nop = nc.sync.nop()
                add_dep_helper(nop.ins, td.ins, sync=True,
                               reason="drain pre-absorber")
                if prev_nop is not None:
                    add_dep_helper(nop.ins, prev_nop.ins, sync=False,
                                   reason="keep nop chain ordered")
                prev_nop = nop
    return nc


def make_in_maps(x: np.ndarray, patts: np.ndarray) -> list[dict[str, np.ndarray]]:
    x = np.ascontiguousarray(x, dtype=np.float32)
    patts = np.ascontiguousarray(patts, dtype=np.float32)
    # lhs columns: col = q*128 + b*32 + k  <->  pattern p = b*8 + q
    pf = patts.transpose(1, 2, 0)                     # (d, P, K)
    p2f = (patts * patts).sum(axis=1).T               # (P, K)
    lhs = np.empty((KAUG, P * K), np.float32)
    cols = np.arange(P * K)
    q, b, k = cols // 128, (cols % 128) // 32, cols % 32
    p = b * 8 + q
    lhs[:D, :] = -2.0 * pf[:, p, k]
    lhs[D, :] = p2f[p, k]
    lhs[D + 1, :] = 1.0
    lhs16 = lhs.astype(np.float16)

    in_maps = []
    for c in range(NCORES):
        xs = x[c * NLOC:(c + 1) * NLOC]                          # (8, 8, 128)
        xf = xs.transpose(1, 0, 2).reshape(D, NLOC * W)          # [d, (n w)]
        x2 = (xs * xs).sum(axis=1).reshape(1, NLOC * W)
        rhs = np.concatenate(
            [xf, np.ones((1, NLOC * W), np.float32), x2], axis=0)
        in_maps.append({"inp": np.concatenate(
            [lhs16, rhs.astype(np.float16)], axis=1)})
    return in_maps


_program_cache: bass.Bass | None = None


def kernel(x: np.ndarray, patts: np.ndarray) -> np.ndarray:
    global _program_cache
    if _program_cache is None:
        _program_cache = build_program()
    nc = _program_cache
    in_maps = make_in_maps(x, patts)
    res = run_bass_kernel_spmd(nc, in_maps, list(range(NCORES)))
    return np.concatenate([r["out"] for r in res.results], axis=0)


if __name__ == "__main__":
    rng = np.random.default_rng(0)
    x = rng.standard_normal((N, D, W), dtype=np.float32)
    patts = rng.standard_normal((K, D, P), dtype=np.float32)
    out = kernel(x, patts)
    print(out.shape, out.dtype)
